# revision 10
# baseline (speedup 1.0000x reference)
"""Trainium2 Bass kernel for nn_Agent_50500225466537 (retrieval_knn GCN agent).

Strategy (8-core SPMD, 1D row-shard of the N=8192 node dim):
  - Host passes each core its column-slice of (A + I).T for both graphs
    (pure layout prep so the contraction dim lands on SBUF partitions),
    int32, cast to fp8e4 inline by SWDGE DMA (0/1/2 are exact in fp8).
  - Device, per graph: column sums of A_hat (DVE free-dim reduces +
    AllReduce / ReduceScatter), Md = (X @ W1) * (64/d) in fp8, the big
    propagation S^T = Md^T @ AhT as fp8 DoubleRow matmuls with Md
    stationary, sigmoid epilogue -> h^T, u = h @ W2, AllGather(u), and
    for graph y the layer-2 matvec G_y = sigmoid(((A+I) @ (u/d)) / d + b2)
    also as DoubleRow matmuls (the 2^6 scaling cancels via the epilogue
    reciprocal scale).
  - Graph y streams first; its whole compute chain hides under graph x's
    stream. Only x's tail (AllReduce + matmul + u_x) is exposed.
  - Host does only the O(N) tail: G_x[index_x] dot product, cosine
    top-11 over G_y, and the final (1,2) softmax.
"""
import os
import sys

for _p in ("/opt/trn_rl_repo", "/root/.axon_site/_ro/trn_rl_repo"):
    if os.path.isdir(_p) and _p not in sys.path:
        sys.path.insert(0, _p)

import numpy as np

import concourse.bacc as bacc
from concourse import bass_utils, mybir, tile

N = 8192
NCORES = 8
R = N // NCORES          # rows per core: 1024
PB = 128                 # partition block
KB = N // PB             # 64 k-blocks
KB2 = KB // 2            # 32 k-block pairs (fp8 DoubleRow)
D = 256                  # feature dim (= hidden dim)
RKB = R // PB            # 8 i-tiles per 1024 chunk
EPS = 1e-8
K_OPP = 11
MDS = 64.0               # fp8 scale for Md / v (power of two, exact)

F32 = mybir.dt.float32
BF16 = mybir.dt.bfloat16
FP8 = mybir.dt.float8e4
I32 = mybir.dt.int32
AX = mybir.AxisListType.X
AF = mybir.ActivationFunctionType
MUL = mybir.AluOpType.mult
ADD = mybir.AluOpType.add
BYPASS = mybir.AluOpType.bypass
DR = mybir.MatmulPerfMode.DoubleRow
GROUPS = [list(range(NCORES))]


class _G:
    """Per-graph emission state."""
    pass


def _transpose_p_f(nc, out_ap, in_ap, pdim, fdim):
    """out[f, p] = in[p, f] via DVE 32x32 block transposes."""
    for bp in range(pdim // 32):
        for bf in range(fdim // 32):
            nc.vector.transpose(
                out_ap[bf * 32:(bf + 1) * 32, bp * 32:(bp + 1) * 32],
                in_ap[bp * 32:(bp + 1) * 32, bf * 32:(bf + 1) * 32],
            )


def _stage_stream(nc, P, g):
    """Stream A_hat^T shard (int32 -> fp8 pair tiles) + column sums,
    then kick the d collectives."""
    g.at = []
    g.d_part = P.small2.tile([PB, KB], F32, tag="d_part", name="d_part")
    dummy = P.small2.tile([PB, R], BF16, tag="cs_dummy", name="cs_dummy")
    for kb2 in range(KB2):
        t = P.at.tile([PB, 2, R], FP8, tag="at", name="at")
        nc.sync.dma_start(
            t[:], g.ahT[kb2 * 256:(kb2 + 1) * 256, :]
            .rearrange("(ko p) i -> p ko i", p=PB))
        for ko in range(2):
            kb = 2 * kb2 + ko
            dsl = g.d_part[:, kb:kb + 1]
            if kb % 2 == 0:
                nc.vector.reduce_sum(dsl, t[:, ko, :], axis=AX)
            else:
                nc.scalar.activation(dummy[:], t[:, ko, :], AF.Copy,
                                     accum_out=dsl)
        g.at.append(t)

    dT = P.small2.tile([KB, PB], F32, tag="dT", name="dT")
    _transpose_p_f(nc, dT[:], g.d_part[:], PB, KB)
    g.d_in = P.dram.tile([N], F32, name="d_in")
    g.d_ar = P.dram.tile([N], F32, name="d_ar")
    g.d_rs = P.dram.tile([R], F32, name="d_rs")
    nc.sync.dma_start(g.d_in[:], dT[:])


def _stage_recip(nc, P, g):
    """Load reduced d back; compute reciprocals (plain and 64x-scaled)."""
    dAT = P.small2.tile([KB, PB], F32, tag="dAT", name="dAT")
    nc.sync.dma_start(dAT[:], g.d_ar[:])
    d128 = P.small2.tile([PB, KB], F32, tag="d128", name="d128")
    _transpose_p_f(nc, d128[:], dAT[:], KB, PB)
    g.recip128 = P.small2.tile([PB, KB], F32, tag="recip128", name="recip128")
    nc.vector.reciprocal(g.recip128[:], d128[:])
    g.recip128s = P.small2.tile([PB, KB], F32, tag="recip128s", name="recip128s")
    nc.vector.tensor_scalar_mul(g.recip128s[:], g.recip128[:], MDS)

    g.recip_loc = P.small1.tile([1, R], F32, tag="recip_loc",
                                name="recip_loc")
    nc.sync.dma_start(g.recip_loc[:], g.d_rs[:])
    nc.vector.reciprocal(g.recip_loc[:], g.recip_loc[:])
    # rb = broadcast of recip_loc / 64  (undoes the fp8 Md scaling)
    g.rb = P.small1.tile([PB, R], F32, tag="rb", name="rb")
    nc.vector.tensor_scalar_mul(g.recip_loc[:], g.recip_loc[:], 1.0 / MDS)
    nc.gpsimd.partition_broadcast(g.rb[:], g.recip_loc[:])


def _stage_xw1(nc, P, g, scaled):
    """Md[k, n] = (X @ W1)[k, n] (* 64/d_k if scaled) -> fp8 pair tiles."""
    g.md = []
    embc = []
    for it in range(KB):
        kb2, ko = divmod(it, 2)
        ic, il = divmod(it, RKB)
        if il == 0:
            embc = []
            for dc in range(2):
                e = P.emb.tile([PB, R], BF16, tag=f"emb{dc}", name=f"emb{dc}")
                nc.sync.dma_start(
                    e[:], g.embT[dc * PB:(dc + 1) * PB, ic * R:(ic + 1) * R])
                embc.append(e)
        ps = P.ps_xw1.tile([PB, D], F32, tag="ps_xw1", name="ps_xw1")
        for dc in range(2):
            nc.tensor.matmul(ps[:], embc[dc][:, il * PB:(il + 1) * PB],
                             P.W1bf[:, dc * D:(dc + 1) * D],
                             start=(dc == 0), stop=(dc == 1))
        if ko == 0:
            m = P.md.tile([PB, 2, D], FP8, tag="md", name="md")
            g.md.append(m)
        m = g.md[kb2]
        if scaled:
            nc.vector.tensor_scalar_mul(m[:, ko, :], ps[:],
                                        g.recip128s[:, it:it + 1])
        else:
            nc.vector.tensor_copy(m[:, ko, :], ps[:])


def _stage_md_scale(nc, P, g):
    """In-place scale of unscaled fp8 Md tiles by 64/d_k."""
    for it in range(KB):
        kb2, ko = divmod(it, 2)
        m = g.md[kb2][:, ko, :]
        if it % 2 == 0:
            nc.vector.tensor_scalar_mul(m, m, g.recip128s[:, it:it + 1])
        else:
            nc.scalar.activation(m, m, AF.Copy,
                                 scale=g.recip128s[:, it:it + 1])


def _stage_bigmm(nc, P, g):
    """S^T = Md^T @ AhT (DoubleRow, accumulate over kb2), then
    h^T = sigmoid(S^T * recip_i / 64 + b1), u = h @ W2."""
    psS = [P.ps_s.tile([PB, 512], F32, tag="psS", name="psS") for _ in range(4)]
    # bank-contiguous runs: 32 back-to-back MMs per PSUM bank keep the
    # PE busy-window dense (avoids the bank-cycling HAM oscillation)
    for nh in range(2):
        for ih in range(2):
            for kb2 in range(KB2):
                nc.tensor.matmul(psS[nh * 2 + ih][:],
                                 g.md[kb2][:, :, nh * PB:(nh + 1) * PB],
                                 g.at[kb2][:, :, ih * 512:(ih + 1) * 512],
                                 start=(kb2 == 0), stop=(kb2 == KB2 - 1),
                                 perf_mode=DR)

    hT = [P.small1.tile([PB, R], BF16, tag=f"hT{nh}", name=f"hT{nh}")
          for nh in range(2)]
    for nh in range(2):
        for ih in range(2):
            p = psS[nh * 2 + ih]
            nc.vector.tensor_mul(p[:], p[:], g.rb[:, ih * 512:(ih + 1) * 512])
            nc.scalar.activation(hT[nh][:, ih * 512:(ih + 1) * 512], p[:],
                                 AF.Sigmoid, bias=P.b1_2[:, nh:nh + 1])

    psu = [P.ps_small.tile([1, 512], F32, tag="ps_small", name="ps_small")
           for _ in range(2)]
    for ih in range(2):
        for nh in range(2):
            nc.tensor.matmul(psu[ih][:], P.W2bf[:, nh:nh + 1],
                             hT[nh][:, ih * 512:(ih + 1) * 512],
                             start=(nh == 0), stop=(nh == 1))
    g.u_loc = P.small1.tile([1, R], F32, tag="u_loc", name="u_loc")
    for ih in range(2):
        nc.scalar.activation(g.u_loc[:, ih * 512:(ih + 1) * 512], psu[ih][:],
                             AF.Copy)
    nc.sync.dma_start(g.u_out, g.u_loc[:])


def _stage_matvec(nc, P, g):
    """AllGather u; v = u * 64/d (fp8); w = (A+I) @ v (DoubleRow);
    G = sigmoid(w * recip_i / 64 + b2)."""
    u_in = P.dram.tile([R], F32, name="u_in")
    u_ag = P.dram.tile([N], F32, name="u_ag")
    nc.sync.dma_start(u_in[:], g.u_loc[:])
    nc.gpsimd.collective_compute("AllGather", BYPASS, replica_groups=GROUPS,
                                 ins=[u_in.opt()], outs=[u_ag.opt()])
    uAT = P.small2.tile([KB, PB], F32, tag="uAT", name="uAT")
    nc.sync.dma_start(uAT[:], u_ag[:])
    u128 = P.small2.tile([PB, KB], F32, tag="u128", name="u128")
    _transpose_p_f(nc, u128[:], uAT[:], KB, PB)
    # v8[p, kb, 0] = u_k * 64/d_k in fp8; pair stride 16B for DoubleRow lhsT
    v8 = P.small2.tile([PB, KB, 16], FP8, tag="v8", name="v8")
    nc.vector.tensor_mul(v8[:, :, 0:1], u128[:], g.recip128s[:])

    psg = [P.ps_small.tile([1, 512], F32, tag="ps_small", name="ps_small")
           for _ in range(2)]
    for ih in range(2):
        for kb2 in range(KB2):
            nc.tensor.matmul(psg[ih][:], v8[:, 2 * kb2:2 * kb2 + 2, 0:1],
                             g.at[kb2][:, :, ih * 512:(ih + 1) * 512],
                             start=(kb2 == 0), stop=(kb2 == KB2 - 1),
                             perf_mode=DR)
    G_sb = P.small1.tile([1, R], F32, tag="G_sb", name="G_sb")
    for ih in range(2):
        p = psg[ih]
        nc.vector.tensor_mul(p[:], p[:], g.rb[0:1, ih * 512:(ih + 1) * 512])
        nc.scalar.activation(G_sb[:, ih * 512:(ih + 1) * 512], p[:],
                             AF.Sigmoid, bias=P.b2sb[:])
    nc.sync.dma_start(g.G_out, G_sb[:])


_CACHED_NC = None


def _build_program():
    global _CACHED_NC
    if _CACHED_NC is not None:
        return _CACHED_NC
    nc = bacc.Bacc("TRN2", target_bir_lowering=False, debug=False,
                   enable_asserts=False, num_devices=NCORES)

    gy = _G()
    gx = _G()
    gy.tag, gx.tag = "y", "x"
    gy.cs_split, gx.cs_split = True, False
    gy.warm, gx.warm = True, False
    gx.ahT = nc.dram_tensor("ahT_x", [N, R], FP8, kind="ExternalInput").ap()
    gy.ahT = nc.dram_tensor("ahT_y", [N, R], FP8, kind="ExternalInput").ap()
    gx.embT = nc.dram_tensor("embT_x", [D, N], BF16, kind="ExternalInput").ap()
    gy.embT = nc.dram_tensor("embT_y", [D, N], BF16, kind="ExternalInput").ap()
    W1_in = nc.dram_tensor("W1", [D, D], F32, kind="ExternalInput").ap()
    b1_in = nc.dram_tensor("b1_2", [PB, 2], F32, kind="ExternalInput").ap()
    W2_in = nc.dram_tensor("W2_2", [PB, 2], F32, kind="ExternalInput").ap()
    b2_in = nc.dram_tensor("b2", [1, 1], F32, kind="ExternalInput").ap()

    gx.u_out = nc.dram_tensor("u_x", [1, R], F32, kind="ExternalOutput").ap()
    gy.u_out = nc.dram_tensor("u_y", [1, R], F32, kind="ExternalOutput").ap()
    gy.G_out = nc.dram_tensor("G_y", [1, R], F32, kind="ExternalOutput").ap()
    gx.d_out = nc.dram_tensor("d_x", [N], F32, kind="ExternalOutput").ap()
    gy.d_out = nc.dram_tensor("d_y", [N], F32, kind="ExternalOutput").ap()

    with tile.TileContext(nc) as tc:
        P = _G()
        import contextlib
        with contextlib.ExitStack() as st:
            P.at = st.enter_context(tc.tile_pool(name="at", bufs=2 * KB2))
            P.md = st.enter_context(tc.tile_pool(name="md", bufs=KB2 + 1))
            P.emb = st.enter_context(tc.tile_pool(name="emb", bufs=6))
            P.small1 = st.enter_context(tc.tile_pool(name="small1", bufs=1))
            P.small2 = st.enter_context(tc.tile_pool(name="small2", bufs=2))
            P.w = st.enter_context(tc.tile_pool(name="w", bufs=1))
            P.ps_s = st.enter_context(tc.tile_pool(name="ps_s", bufs=4, space="PSUM"))
            P.ps_xw1 = st.enter_context(tc.tile_pool(name="ps_xw1", bufs=2, space="PSUM"))
            P.ps_small = st.enter_context(tc.tile_pool(name="ps_small", bufs=2, space="PSUM"))
            P.dram = st.enter_context(tc.tile_pool(name="dram", bufs=16, space="DRAM"))

            # small persistent weights
            P.W1bf = P.w.tile([PB, 2 * D], BF16, tag="W1bf", name="W1bf")
            for dc in range(2):
                nc.gpsimd.dma_start(P.W1bf[:, dc * D:(dc + 1) * D],
                                    W1_in[dc * PB:(dc + 1) * PB, :])
            P.b1_2 = P.w.tile([PB, 2], F32, tag="b1_2", name="b1_2")
            nc.sync.dma_start(P.b1_2[:], b1_in)
            P.W2bf = P.w.tile([PB, 2], BF16, tag="W2bf", name="W2bf")
            nc.gpsimd.dma_start(P.W2bf[:], W2_in)
            P.b2sb = P.w.tile([1, 1], F32, tag="b2sb", name="b2sb")
            nc.sync.dma_start(P.b2sb[:], b2_in)

            # emission order sets scheduler priority: y stream, x stream,
            # then y's whole chain (hidden under x stream), then x's tail.
            _stage_stream(nc, P, gy)
            _stage_stream(nc, P, gx)
            # CC stream order: the AllReduces gate the matmul chains; the
            # ReduceScatters only gate the (later) epilogues
            for g in (gy, gx):
                nc.gpsimd.collective_compute(
                    "AllReduce", ADD, replica_groups=GROUPS,
                    ins=[g.d_in.opt()], outs=[g.d_ar.opt()])
            for g in (gy, gx):
                nc.gpsimd.collective_compute(
                    "ReduceScatter", ADD, replica_groups=GROUPS,
                    ins=[g.d_in.opt()], outs=[g.d_rs.opt()])
                nc.sync.dma_start(g.d_out, g.d_ar[:])
            _stage_recip(nc, P, gy)
            _stage_xw1(nc, P, gy, scaled=True)
            _stage_bigmm(nc, P, gy)
            _stage_matvec(nc, P, gy)
            _stage_xw1(nc, P, gx, scaled=False)
            _stage_recip(nc, P, gx)
            _stage_md_scale(nc, P, gx)
            _stage_bigmm(nc, P, gx)

    nc.compile()
    _CACHED_NC = nc
    return nc


def _prep_in_maps(A_x, A_y, first_embeddings, second_embeddings, W1, b1, W2, b2):
    import ml_dtypes

    def shards(A):
        AhT = np.ascontiguousarray(A.T).astype(np.int8, copy=False)
        AhT[np.arange(N), np.arange(N)] += 1
        AhT = AhT.astype(ml_dtypes.float8_e4m3fn)
        return [np.ascontiguousarray(AhT[:, c * R:(c + 1) * R])
                for c in range(NCORES)]

    shx = shards(A_x)
    shy = shards(A_y)
    embT_x = np.ascontiguousarray(first_embeddings.T).astype(ml_dtypes.bfloat16)
    embT_y = np.ascontiguousarray(second_embeddings.T).astype(ml_dtypes.bfloat16)
    b1_2 = np.ascontiguousarray(b1.reshape(2, PB).T)
    W2_2 = np.ascontiguousarray(W2[:, 0].reshape(2, PB).T)
    b2_in = b2.reshape(1, 1)
    return [
        dict(ahT_x=shx[c], ahT_y=shy[c], embT_x=embT_x, embT_y=embT_y,
             W1=W1, b1_2=b1_2, W2_2=W2_2, b2=b2_in)
        for c in range(NCORES)
    ]


def _sigmoid(x):
    return 1.0 / (1.0 + np.exp(-x))


def kernel(A_x, A_y, first_embeddings, second_embeddings, W1, b1, W2, b2,
           W_h, W_f, W_p, bias_h, index_x, index_y):
    A_x = np.asarray(A_x)
    A_y = np.asarray(A_y)
    first_embeddings = np.asarray(first_embeddings, dtype=np.float32)
    second_embeddings = np.asarray(second_embeddings, dtype=np.float32)
    W1 = np.asarray(W1, dtype=np.float32)
    b1 = np.asarray(b1, dtype=np.float32)
    W2 = np.asarray(W2, dtype=np.float32)
    b2 = np.asarray(b2, dtype=np.float32)
    W_h = np.asarray(W_h, dtype=np.float32)
    W_f = np.asarray(W_f, dtype=np.float32)
    W_p = np.asarray(W_p, dtype=np.float32)
    bias_h = np.asarray(bias_h, dtype=np.float32)
    ix = int(index_x)
    iy = int(index_y)

    nc = _build_program()
    in_maps = _prep_in_maps(A_x, A_y, first_embeddings, second_embeddings,
                            W1, b1, W2, b2)
    res = bass_utils.run_bass_kernel_spmd(nc, in_maps, core_ids=list(range(NCORES)))
    results = res.results

    u_x = np.concatenate([results[c]["u_x"][0] for c in range(NCORES)])
    G_y_full = np.concatenate([results[c]["G_y"][0] for c in range(NCORES)])
    d_x = results[0]["d_x"]

    # ---- host tail (tiny O(N) ops), fp32 like the reference ----
    row = A_x[ix].astype(np.float32)
    row[ix] += 1.0
    pre = np.float32(row @ (u_x / d_x)) / d_x[ix] + b2[0]
    g_x = _sigmoid(np.float32(pre))
    g_y = G_y_full[iy]

    cat = np.array([[g_x], [g_y]], dtype=np.float32)        # (2, 1)
    h = _sigmoid(W_h @ cat + bias_h)                        # (1, 1)
    f = np.exp(g_x * W_f * g_y)                             # (1, 1)

    # cosine-similarity top-k over G_y (C = 1)
    num = G_y_full * g_y
    ng = np.maximum(np.abs(G_y_full), np.float32(EPS))
    nv = np.maximum(np.abs(g_y), np.float32(EPS))
    sims = num / (ng * nv)
    idx = np.argsort(-sims, kind="stable")[:K_OPP]
    opp = G_y_full[idx]
    f_oppo = np.float32(np.sum(np.exp(g_x * W_f[0, 0] * opp)))

    I_val = f / f_oppo                                      # (1, 1)
    z = W_p @ np.concatenate([h, I_val], axis=1)            # (1, 2)
    zs = z - z.max(axis=1, keepdims=True)
    ez = np.exp(zs)
    policy = ez / ez.sum(axis=1, keepdims=True)
    return policy.astype(np.float32)
